# revision 19
# baseline (speedup 1.0000x reference)
"""DecoderFeatureFuser3D kernel v4 for Trainium2 (8 NeuronCores, data-parallel).

Math (per batch b):
    g2d  = bilinear_sample(feat_2d[b], xy[b])          # [C2d, N]
    cat  = concat([g2d, feat_3d[b]])                   # [C2d+C3d, N]
    y    = leaky_relu(W @ cat + b, 0.1)                # [C3d, N]

v4 strategy (per core: batch b = core//2, point-half h = core%2):
  - Host builds a row-pair-interleaved image f2q[r*W+x] = [px(r,x), px(r+1,x)]
    (fp32, [H-1)*W, 2C]) so ONE gather token of 2KB (elem_step = 1 entry)
    fetches all 4 bilinear corners of a point: [t0, b0, t1, b1].
  - The gather reads f2q (an ExternalInput) directly -- no internal staging
    copy. Points are host-sorted by (y0, x0) for HBM locality; the inverse
    permutation is applied on the host after the run.
  - 8 chunks x 1024 points, all on SWDGE queue 0: single-queue FIFO makes
    chunk completions ordered, so interp pipelines behind the gather stream.
  - DVE does the 7-op interp per chunk (4 mult + 3 add, weights broadcast
    along channels via stride-0 APs) in point-major layout.
  - PE: transpose fused -> [ch, pt] psum, ACT copy to SBUF, then
    y = WaT^T @ g2dT + WbT^T @ f3d in PSUM (fp32).
  - Bias+leaky on Scalar (t01 = 0.1*yp + 0.1*b) + Pool stt (max(yp+b, t01)),
    keeping the DVE free for interp.
"""

import numpy as np

B = 4
C = 128
H, W_IMG = 96, 160
N = 16384
N_CORES = 8
NPC = N // 2             # 8192 points per core
NGROUPS = NPC // 128     # 64 groups of 128 points
NCHUNKS = 8
CPTS = NPC // NCHUNKS    # 1024 points per gather chunk
CG = CPTS // 128         # 8 groups per chunk
NPAIR = (H - 1) * W_IMG  # 15200 row-pair tokens
NEG_SLOPE = 0.1
BANDS = None             # kept for test.py compatibility (unused in v4)

_CACHE = {}


def _build_program(bands=None):
    import concourse.bass as bass
    import concourse.bacc as bacc
    import concourse.mybir as mybir
    import concourse.tile as tile

    f32 = mybir.dt.float32
    i16 = mybir.dt.int16

    nc = bacc.Bacc(
        "TRN2",
        target_bir_lowering=False,
        debug=False,
        enable_asserts=False,
        num_devices=N_CORES,
        num_swdge_queues=1,
    )

    ins = {
        "f2q": nc.dram_tensor("f2q", [NPAIR, 2 * C], f32, kind="ExternalInput").ap(),
        "f3d": nc.dram_tensor("f3d", [C, NPC], f32, kind="ExternalInput").ap(),
        "wat": nc.dram_tensor("wat", [C, C], f32, kind="ExternalInput").ap(),
        "wbt": nc.dram_tensor("wbt", [C, C], f32, kind="ExternalInput").ap(),
        "bias": nc.dram_tensor("bias", [C, 1], f32, kind="ExternalInput").ap(),
        "bias01": nc.dram_tensor("bias01", [C, 1], f32, kind="ExternalInput").ap(),
        "idx": nc.dram_tensor("idx", [128, NPC // 16], i16, kind="ExternalInput").ap(),
        "w4": nc.dram_tensor(
            "w4", [128, 4 * NGROUPS], f32, kind="ExternalInput"
        ).ap(),
    }
    outs = {
        "y": nc.dram_tensor("y", [C, NPC], f32, kind="ExternalOutput").ap(),
    }

    with tile.TileContext(nc) as tc:
        build_device_kernel(tc, outs, ins)

    nc.compile()
    return nc


def build_device_kernel(tc, outs, ins):
    from contextlib import ExitStack

    import concourse.bass as bass
    import concourse.mybir as mybir
    from concourse.masks import make_identity

    f32 = mybir.dt.float32
    nc = tc.nc
    alu = mybir.AluOpType
    act_fn = mybir.ActivationFunctionType

    y = outs["y"]

    with ExitStack() as ctx:
        const = ctx.enter_context(tc.tile_pool(name="const", bufs=1))
        big = ctx.enter_context(tc.tile_pool(name="big", bufs=1))
        gat = ctx.enter_context(tc.tile_pool(name="gat", bufs=4))
        fus = ctx.enter_context(tc.tile_pool(name="fus", bufs=2))
        ysb = ctx.enter_context(tc.tile_pool(name="ysb", bufs=2))
        g2sb = ctx.enter_context(tc.tile_pool(name="g2sb", bufs=3))
        psg_p = ctx.enter_context(tc.tile_pool(name="psg", bufs=2, space="PSUM"))
        yp_p = ctx.enter_context(tc.tile_pool(name="yp", bufs=4, space="PSUM"))

        # ---- idx through the Pool engine's own SWDGE queue: FIFO-ordered
        # right before the gathers on the same ring, so the first gather
        # is not held hostage by the HWDGE const/f3d preamble (~12us).
        idx_sb = const.tile([128, NPC // 16], ins["idx"].dtype, tag="idx")
        nc.gpsimd.dma_start(out=idx_sb, in_=ins["idx"])

        ident = const.tile([128, 128], f32)
        make_identity(nc, ident)

        # pair-token view of the full image: token i = f2q[i : i+2, :] (2KB)
        tok = bass.AP(
            ins["f2q"].tensor, ins["f2q"].offset, [[2 * C, NPAIR - 1], [1, 4 * C]]
        )

        # ---- all gathers up front (program order on gpsimd = queue order).
        # Desc-gen is the serial cost (~8ns/idx on the Pool engine); one
        # 2KB token per point minimizes the idx count.
        gq_tiles = []
        for ci in range(NCHUNKS):
            isl = slice(ci * CPTS // 16, (ci + 1) * CPTS // 16)
            gq = gat.tile([128, CG, 4 * C], f32, tag="gq")
            nc.gpsimd.dma_gather(
                out_ap=gq[:],
                in_ap=tok,
                idxs_ap=idx_sb[:, isl],
                num_idxs=CPTS,
                num_idxs_reg=CPTS,
                elem_size=4 * C,
                elem_step=2 * C,
                single_packet=True,
                queue_num=0,
            )
            gq_tiles.append(gq)

        # packed weights: w4[p, 4*g + k], k in token order [t0, b0, t1, b1]
        w4_sb = const.tile([128, 4 * NGROUPS], f32, tag="w4")
        nc.sync.dma_start(out=w4_sb, in_=ins["w4"])

        wat_sb = const.tile([C, C], f32)
        nc.sync.dma_start(out=wat_sb, in_=ins["wat"])
        wbt_sb = const.tile([C, C], f32)
        nc.sync.dma_start(out=wbt_sb, in_=ins["wbt"])
        b_sb = const.tile([C, 1], f32)
        nc.sync.dma_start(out=b_sb, in_=ins["bias"])
        b01_sb = const.tile([C, 1], f32)
        nc.sync.dma_start(out=b01_sb, in_=ins["bias01"])

        # f3d + y stream on the Scalar HWDGE ring so the Sync ring stays free.
        # 3-dim APs chop it into 4KB descriptors so it interleaves fairly with
        # the gather/idx descriptors instead of clogging engines with 32KB ones.
        f3d_sb = big.tile([C, NPC], f32)
        f3a = f3d_sb[:]
        f3d_out = bass.AP(
            f3a.tensor, f3a.offset, [f3a.ap[0], [1024, NPC // 1024], [1, 1024]]
        )
        f3i = ins["f3d"]
        f3d_in = bass.AP(
            f3i.tensor, f3i.offset, [f3i.ap[0], [1024, NPC // 1024], [1, 1024]]
        )
        nc.scalar.dma_start(out=f3d_out, in_=f3d_in)

        def dim4(sl, k, c):
            """Reshape a [128, CG, k*c] slice AP to [128, CG, k, c]."""
            return bass.AP(
                sl.tensor, sl.offset, [sl.ap[0], sl.ap[1], [c, k], [1, c]]
            )

        for ci in range(NCHUNKS):
            gq = gq_tiles[ci]
            y_sb = ysb.tile([C, CPTS], f32, tag="ych")

            # ---- interpolate: fused = sum_k w_k * v_k in 3 wide DVE ops
            # token layout per point: [t0(0:C), b0(C:2C), t1(2C:3C), b1(3C:4C)]
            fw = fus.tile([128, CG, 4 * C], f32, tag="fw")
            wsl = w4_sb[:, 4 * ci * CG : 4 * (ci + 1) * CG]
            wb = bass.AP(
                wsl.tensor, wsl.offset, [wsl.ap[0], [4, CG], [1, 4], [0, C]]
            )
            nc.vector.tensor_tensor(
                out=dim4(fw[:, :, :], 4, C),
                in0=dim4(gq[:, :, :], 4, C),
                in1=wb,
                op=alu.mult,
            )
            t2 = fus.tile([128, CG, 2 * C], f32, tag="t2", bufs=1)
            nc.vector.tensor_tensor(
                out=dim4(t2[:, :, :], 2, C),
                in0=dim4(fw[:, :, 0 : 2 * C], 2, C),
                in1=dim4(fw[:, :, 2 * C : 4 * C], 2, C),
                op=alu.add,
            )
            fused = fus.tile([128, CG, C], f32, tag="fused")
            nc.vector.tensor_tensor(
                out=fused, in0=t2[:, :, 0:C], in1=t2[:, :, C : 2 * C], op=alu.add
            )

            # ---- per 512-pt tile: transpose, copy, matmuls, bias+leaky
            for q in range(CG * 128 // 512):
                psg = psg_p.tile([128, 512], f32, tag="psg")
                for g4 in range(4):
                    g = q * 4 + g4
                    nc.tensor.matmul(
                        out=psg[:, g4 * 128 : (g4 + 1) * 128],
                        lhsT=fused[:, g, :],
                        rhs=ident[:],
                        is_transpose=True,
                        start=True,
                        stop=True,
                    )
                g2t = g2sb.tile([128, 512], f32, tag="g2t")
                nc.scalar.activation(out=g2t, in_=psg, func=act_fn.Copy)

                yp = yp_p.tile([128, 512], f32, tag="yp")
                p0 = ci * CPTS + q * 512
                nc.tensor.matmul(
                    out=yp, lhsT=wat_sb[:], rhs=g2t[:], start=True, stop=False
                )
                nc.tensor.matmul(
                    out=yp,
                    lhsT=wbt_sb[:],
                    rhs=f3d_sb[:, p0 : p0 + 512],
                    start=False,
                    stop=True,
                )
                # t01 = 0.1*yp + 0.1*b (Scalar); y = max(yp + b, t01) (DVE)
                q0 = q * 512
                t01 = g2sb.tile([128, 512], f32, tag="t01")
                nc.scalar.activation(
                    out=t01,
                    in_=yp,
                    func=act_fn.Identity,
                    scale=NEG_SLOPE,
                    bias=b01_sb[:, 0:1],
                )
                nc.vector.scalar_tensor_tensor(
                    out=y_sb[:, q0 : q0 + 512],
                    in0=yp,
                    scalar=b_sb[:, 0:1],
                    in1=t01,
                    op0=alu.add,
                    op1=alu.max,
                )

            o0 = ci * CPTS
            nc.scalar.dma_start(out=y[:, o0 : o0 + CPTS], in_=y_sb)


def _host_prep(xy, feat_2d, feat_3d, W, b):
    """Shard + repack inputs for the 8 cores. Returns (in_maps, perms)."""
    xy = np.asarray(xy, dtype=np.float32)
    feat_2d = np.asarray(feat_2d, dtype=np.float32)
    feat_3d = np.asarray(feat_3d, dtype=np.float32)
    W = np.asarray(W, dtype=np.float32)
    b = np.asarray(b, dtype=np.float32)

    wat = np.ascontiguousarray(W[:, :C].T)
    wbt = np.ascontiguousarray(W[:, C:].T)
    bvec = np.ascontiguousarray(b.reshape(C, 1))
    b01vec = np.ascontiguousarray((np.float32(NEG_SLOPE) * b).reshape(C, 1))

    # row-pair interleaved image: f2q[r*W+x] = [px(r,x), px(r+1,x)]
    f2qs = []
    for bb in range(B):
        ft = np.ascontiguousarray(feat_2d[bb].transpose(1, 2, 0))  # [H, W, C]
        f2q = np.concatenate([ft[:-1], ft[1:]], axis=2)  # [H-1, W, 2C]
        f2qs.append(np.ascontiguousarray(f2q.reshape(NPAIR, 2 * C)))

    in_maps = []
    perms = []
    for core in range(N_CORES):
        bb, h = divmod(core, 2)
        sl = slice(h * NPC, (h + 1) * NPC)
        x = xy[bb, 0, sl]
        v = xy[bb, 1, sl]

        x0 = np.minimum(np.floor(x), W_IMG - 2)
        y0 = np.minimum(np.floor(v), H - 2)
        ix = np.clip(x0, 0, None).astype(np.int64)
        iy = np.clip(y0, 0, None).astype(np.int64)

        # sort points by (y0, x0) for gather locality
        perm = np.lexsort((ix, iy))
        x = x[perm]; v = v[perm]
        x0 = x0[perm]; y0 = y0[perm]
        ix = ix[perm]; iy = iy[perm]
        perms.append(perm)

        wx1 = x - x0
        wy1 = v - y0
        wx0 = np.float32(1.0) - wx1
        wy0 = np.float32(1.0) - wy1

        idx = iy * W_IMG + ix  # row-pair token index, < 15200

        # packed weights [128, NGROUPS*4]: w4[p, 4g+k], k-order [t0, b0, t1, b1]
        wk = np.stack(
            [wx0 * wy0, wx0 * wy1, wx1 * wy0, wx1 * wy1], axis=-1
        ).astype(np.float32)  # [NPC, 4]
        w4 = np.ascontiguousarray(
            wk.reshape(NGROUPS, 128, 4).transpose(1, 0, 2).reshape(128, 4 * NGROUPS)
        )

        def wrap16(a):
            w = np.ascontiguousarray(a.astype(np.int16).reshape(NPC // 16, 16).T)
            return np.ascontiguousarray(np.tile(w, (8, 1)))

        in_maps.append(
            {
                "f2q": f2qs[bb],
                "f3d": np.ascontiguousarray(feat_3d[bb, :, sl][:, perm]),
                "wat": wat,
                "wbt": wbt,
                "bias": bvec,
                "bias01": b01vec,
                "idx": wrap16(idx),
                "w4": w4,
            }
        )
    return in_maps, perms


def kernel(xy, feat_2d, feat_3d, W, b):
    from concourse.bass_utils import run_bass_kernel_spmd

    if "nc" not in _CACHE:
        _CACHE["nc"] = _build_program(BANDS)
    nc = _CACHE["nc"]

    in_maps, perms = _host_prep(xy, feat_2d, feat_3d, W, b)
    res = run_bass_kernel_spmd(nc, in_maps, list(range(N_CORES)))

    out = np.empty((B, C, N), dtype=np.float32)
    for core in range(N_CORES):
        bb, h = divmod(core, 2)
        blk = np.empty((C, NPC), dtype=np.float32)
        blk[:, perms[core]] = res.results[core]["y"]
        out[bb, :, h * NPC : (h + 1) * NPC] = blk
    return out
